# revision 27
# baseline (speedup 1.0000x reference)
# Dynamic sparse attention (sliding-window, paged-KV) on 8 TRN2 NeuronCores.
#
# Reference computation (B=2, S=2048, D=1024, H=16, HD=64, window=512):
#   q/k/v = x @ W{q,k,v}.T ; k/v scattered to a paged cache via slot_mapping,
#   gathered back via block_tables ; causal sliding-window attention ;
#   out = attn @ wo.T
#
# Sharding: core c in 0..7 -> batch bi=c//4, head-group hg=c%4 (4 heads each).
# Each core reads only its batch's activations (pre-packed + bf16-cast on
# host) and its head-group's weight slices, and writes a partial output
# (fp16). Host sums the 4 head-group partials per batch and transposes back.
# The paged-cache scatter/gather composes to a single token-gather g
# (identity for the arange block_tables/slot_mapping); it is folded into a
# host-side column gather of x for the K/V projection input.
#
# On-device layout (per core):
#   qT/kT  [128, 2, 2048] bf16   (partition = head-dim pair, free = seq)
#   V^     [128, 16*260] bf16    (keys on partitions; per head 64 V cols +
#                                 a ones column -> PV matmul also accumulates
#                                 the softmax denominator Z for free)
#   scores are computed transposed (S^T[k, q]) per 128-key strip so the
#   exp'd strip feeds the PV matmul directly as the moving operand -- no
#   P transposes. No running-max is needed (scores ~ N(0,1) after 1/8 scale);
#   masked entries are zeroed post-exp by a 0/1 mask multiply on DVE.
#
# Host packs x / weights into chunk-contiguous [128, *] layouts so each
# chunk is one large-line DMA descriptor.

import numpy as np

import concourse.bass as bass
import concourse.tile as tile
from concourse import bacc, mybir
from concourse.bass_utils import run_bass_kernel_spmd

B, S, D, H, HD = 2, 2048, 1024, 16, 64
BLOCK = 16
WINDOW = 512
P = 128
NCORES = 8
HPC = 4          # heads per core
CW = HPC * HD    # per-core projection width = 256
NKB = S // P     # 16 key blocks
NQT = S // 512   # 4 q-tiles of 512
FP32 = mybir.dt.float32
FP16 = mybir.dt.float16
BF16 = mybir.dt.bfloat16
FP8 = mybir.dt.float8e4
HVW = 68         # per-head vhat stride (64 V + 1 ones + 3 pad; 16B-aligned pairs)
VROW = HPC * HVW  # vhat cols per key block
USE_FP8_PV = True  # exp'd probs + V-hat in fp8e4; PV runs DoubleRow (2x PE)


def _strip_width(kb: int) -> int:
    return min(512 + P, S - P * kb)


def _emit(ctx, nc, tc, xc, xcg, wqkv, woTp, mask2, ident, out_dev, single_stream):
    const = ctx.enter_context(tc.tile_pool(name="const", bufs=1))
    xs_pool = ctx.enter_context(tc.tile_pool(name="xs", bufs=3))
    acts = ctx.enter_context(tc.tile_pool(name="acts", bufs=1))
    vt_pool = ctx.enter_context(tc.tile_pool(name="vt", bufs=3))
    strip_pool = ctx.enter_context(tc.tile_pool(name="strips", bufs=40))
    z_pool = ctx.enter_context(tc.tile_pool(name="zch", bufs=4))
    out_pool = ctx.enter_context(tc.tile_pool(name="wo_out", bufs=3))
    psum_mm = ctx.enter_context(tc.tile_pool(name="mm512", bufs=2, space="PSUM"))
    psum_sc = ctx.enter_context(tc.tile_pool(name="pscore", bufs=2, space="PSUM"))
    psum_pv = ctx.enter_context(tc.tile_pool(name="ppv", bufs=2, space="PSUM"))

    # ---- constants (tiles only; DMAs emitted inside proj_chunk(0) so the
    # critical x/weight loads hit the queues first). wqkv is split into
    # per-dt piece tiles so the first proj matmuls start as pieces land. ----
    SDT = FP8 if USE_FP8_PV else BF16  # dtype of exp'd probs, V-hat, mask
    wqkv_p = [const.tile([P, 3 * CW], BF16, name=f"wqkv{dt}") for dt in range(8)]
    woT_s = const.tile([P, 2 * D], BF16, name="woT_s")
    mask_s = const.tile([P, 2 * P], SDT, name="mask_s")
    ident_s = const.tile([P, P], BF16, name="ident_s")

    # ---- per-512-chunk activation tiles: lets attention on early chunks
    # overlap projection of later ones (deps stay per-chunk) ----
    ebias = None
    if USE_FP8_PV:
        ebias = const.tile([P, 1], FP32, name="ebias")
        nc.vector.memset(ebias[:], -2.0)
    qTc = [acts.tile([P, 2 * 512], BF16, name=f"qTc{t}") for t in range(4)]
    kTc = [acts.tile([P, 2 * 512], BF16, name=f"kTc{t}") for t in range(4)]
    vh4 = [acts.tile([P, 4 * VROW], SDT, name=f"vh{t}") for t in range(4)]
    attnT_q = [acts.tile([P, 2 * 512], BF16, name=f"attnT{i}") for i in range(NQT)]
    for t in range(4):
        for h in range(HPC):
            nc.vector.memset(
                vh4[t][:].rearrange("p (kb c) -> p kb c", kb=4)[:, :, HVW * h + 64 : HVW * h + 65],
                1.0,
            )

    def q_ap(h, gc0, gc1):
        # qT slice for global q-cols [gc0, gc1) -- must lie in one chunk
        t = gc0 // 512
        assert gc1 <= 512 * (t + 1)
        ht, hp = h // 2, 64 * (h % 2)
        lo = gc0 - 512 * t
        return qTc[t][hp : hp + 64, 512 * ht + lo : 512 * ht + lo + (gc1 - gc0)]

    def proj_chunk(t):
        with nc.named_scope(f"proj{t}"):
            if t == 0:
                # chunk 0 gates the whole kernel: x pieces and weight pieces
                # are interleaved across both HWDGE queues so the first proj
                # matmuls can start as soon as piece 0 lands; consts follow.
                xs = xs_pool.tile([P, 8 * 512], BF16, tag="xs", name="xs")
                for dt in range(8):
                    xeng = nc.sync if dt % 2 == 0 else nc.scalar
                    xeng.dma_start(
                        out=xs[:, dt * 512 : dt * 512 + 512],
                        in_=xc[0:P, dt * 512 : dt * 512 + 512],
                    )
                    weng = nc.scalar if dt % 2 == 0 else nc.sync
                    weng.dma_start(
                        out=wqkv_p[dt][:],
                        in_=wqkv[:, dt * 3 * CW : (dt + 1) * 3 * CW],
                    )
                nc.scalar.dma_start(out=ident_s[:], in_=ident[:, :])
                nc.scalar.dma_start(out=mask_s[:], in_=mask2[:, :])
                nc.scalar.dma_start(out=woT_s[:], in_=woTp[:, :])
            else:
                xs = xs_pool.tile([P, 8 * 512], BF16, tag="xs", name="xs")
                nc.sync.dma_start(out=xs[:], in_=xc[P * t : P * t + P, :])
            if single_stream:
                xg = xs
            else:
                xg = xs_pool.tile([P, 8 * 512], BF16, tag="xg", name="xg")
                nc.sync.dma_start(out=xg[:], in_=xcg[P * t : P * t + P, :])
            for proj in range(3):
                src = xs if proj == 0 else xg
                for dto in range(2):
                    ps = psum_mm.tile([P, 512], FP32, tag="mm512", name="ps_proj")
                    for dt in range(8):
                        nc.tensor.matmul(
                            ps[:],
                            wqkv_p[dt][:, CW * proj + P * dto : CW * proj + P * dto + P],
                            src[:, dt * 512 : dt * 512 + 512],
                            start=(dt == 0),
                            stop=(dt == 7),
                        )
                    if proj == 0:
                        nc.vector.tensor_copy(out=qTc[t][:, 512 * dto : 512 * dto + 512], in_=ps[:])
                    elif proj == 1:
                        nc.vector.tensor_copy(out=kTc[t][:, 512 * dto : 512 * dto + 512], in_=ps[:])
                    else:
                        vt = vt_pool.tile([P, 512], BF16, tag="vt", name="vt")
                        nc.vector.tensor_copy(out=vt[:], in_=ps[:])
                        for j in range(4):
                            tr = psum_sc.tile([P, P], BF16, tag="score", name="tr")
                            nc.tensor.transpose(tr[:], vt[:, P * j : P * j + P], ident_s[:])
                            vh = vh4[t][:].rearrange(
                                "p (kb g c) -> p kb g c", kb=4, g=HPC
                            )[:, j : j + 1, 2 * dto : 2 * dto + 2, 0:64]
                            nc.any.tensor_copy(
                                out=vh,
                                in_=tr[:].rearrange("p (two c) -> p two c", two=2),
                            )

    # strip pairs: (j, h) -> [128, 2*640] tile holding exp'd scores of key
    # blocks 2j (cols [0,640)) and 2j+1 (cols [640, 640+w)), each in its own
    # q coordinates. Pairing enables fp8 DoubleRow PV matmuls (2 key blocks
    # contracted per pass).
    strip_pairs = {}

    def strip_ap(kb, h, c0, c1):
        # slice of strip kb (local q-cols [c0, c1))
        sp = strip_pairs[(kb // 2, h)]
        off = 640 * (kb % 2)
        return sp[:, off + c0 : off + c1]

    def strip_pair_ap(k0, h, c0, n):
        # [128, 2, n] AP: sub0 = strip k0 cols [c0, c0+n), sub1 = strip k0+1
        # cols [c0-128, c0-128+n) -- the same global q range (offset 128 keys)
        base = strip_pairs[(k0 // 2, h)][:, c0 : c0 + n]
        return bass.AP(base.tensor, base.offset, [base.ap[0], [640 - P, 2], [1, n]])

    def vhat_pair_ap(k0, h):
        base = vh4[k0 // 4][:, VROW * (k0 % 4) + HVW * h : VROW * (k0 % 4) + HVW * h + 65]
        return bass.AP(base.tensor, base.offset, [base.ap[0], [VROW, 2], [1, 65]])

    def strips_qt(qt):
        with nc.named_scope(f"strips_q{qt}"):
            for kb in range(4 * qt, 4 * qt + 4):
                for h in range(HPC):
                    ht, hp = h // 2, 64 * (h % 2)
                    w = _strip_width(kb)
                    n1 = min(512, w)
                    n2 = w - n1
                    ps = psum_sc.tile([P, w], FP32, tag="score", name="ps_sc")
                    lhsT = kTc[kb // 4][hp : hp + 64, 512 * ht + P * (kb % 4) : 512 * ht + P * (kb % 4) + P]
                    # part 1 split at 512-chunk boundaries of qT
                    gc = P * kb
                    while gc < P * kb + n1:
                        end = min(P * kb + n1, (gc // 512 + 1) * 512)
                        nc.tensor.matmul(
                            ps[:, gc - P * kb : end - P * kb],
                            lhsT,
                            q_ap(h, gc, end),
                            start=True,
                            stop=True,
                        )
                        gc = end
                    if n2:
                        nc.tensor.matmul(
                            ps[:, 512 : 512 + n2],
                            lhsT,
                            q_ap(h, P * kb + 512, P * kb + 512 + n2),
                            start=True,
                            stop=True,
                        )
                    if kb % 2 == 0:
                        strip_pairs[(kb // 2, h)] = strip_pool.tile(
                            [P, 2 * 640], SDT, tag="strip", name="strip"
                        )
                    st = strip_ap(kb, h, 0, w)
                    # fp8: bias the exponent down so exp stays within e4m3
                    # range (max |score| ~ 5.5 -> e^3.5 = 33 < 240); the
                    # softmax 1/Z cancels the constant factor exactly
                    nc.scalar.activation(
                        st, ps[:], mybir.ActivationFunctionType.Exp,
                        scale=float(HD) ** -0.5,
                        bias=ebias[:] if USE_FP8_PV else 0.0,
                    )
                    if n2 == P:
                        ed = st.rearrange("p (a c) -> p a c", c=P)[:, 0:5:4, :]
                        nc.vector.tensor_mul(
                            out=ed, in0=ed, in1=mask_s[:].rearrange("p (a c) -> p a c", c=P)
                        )
                    else:
                        nc.vector.tensor_mul(out=st[:, 0:P], in0=st[:, 0:P], in1=mask_s[:, 0:P])
                        if n2:
                            nc.vector.tensor_mul(
                                out=st[:, 512 : 512 + n2],
                                in0=st[:, 512 : 512 + n2],
                                in1=mask_s[:, P : P + n2],
                            )

    def pv_qt(qt):
        with nc.named_scope(f"pv_q{qt}"):
            W0 = 512 * qt
            for h in range(HPC):
                ht, hp = h // 2, 64 * (h % 2)
                pv = psum_pv.tile([65, 512], FP32, tag="pv", name="ps_pv")
                nc.vector.memset(pv[:], 0.0)
                for k0 in range(max(0, 4 * qt - 4), 4 * qt + 4, 2):
                    k1 = k0 + 1
                    c0a = max(P * k0, W0)
                    c0b = min(P * k0 + _strip_width(k0), W0 + 512)
                    c1a = max(P * k1, W0)
                    c1b = min(P * k1 + _strip_width(k1), W0 + 512)
                    a, b = max(c0a, c1a), min(c0b, c1b)
                    if USE_FP8_PV and b > a:
                        nc.tensor.matmul(
                            pv[:, a - W0 : b - W0],
                            vhat_pair_ap(k0, h),
                            strip_pair_ap(k0, h, a - P * k0, b - a),
                            start=False,
                            stop=False,
                            perf_mode=mybir.MatmulPerfMode.DoubleRow,
                            skip_group_check=True,
                        )
                        singles = [(k0, c0a, min(a, c0b)), (k1, max(b, c1a), c1b)]
                    else:
                        singles = [(k0, c0a, c0b), (k1, c1a, c1b)]
                    for kk, sa, sb in singles:
                        if sb <= sa:
                            continue
                        nc.tensor.matmul(
                            pv[:, sa - W0 : sb - W0],
                            vhat_ap(kk, h),
                            strip_ap(kk, h, sa - P * kk, sb - P * kk),
                            start=False,
                            stop=False,
                            skip_group_check=True,
                        )
                zs = z_pool.tile([1, 512], FP32, tag="zs", name="zs")
                nc.scalar.copy(out=zs[:], in_=pv[64:65, :])
                zr = z_pool.tile([1, 512], FP32, tag="zr", name="zr")
                nc.vector.reciprocal_approx_fast(out=zr[:], in_=zs[:])
                zrb = z_pool.tile([64, 512], FP32, tag="zrb", name="zrb")
                nc.gpsimd.partition_broadcast(zrb[:], zr[:])
                nc.vector.tensor_mul(
                    out=attnT_q[qt][hp : hp + 64, 512 * ht : 512 * ht + 512],
                    in0=pv[0:64, :],
                    in1=zrb[:],
                )

    def wo_qt(qt):
        with nc.named_scope(f"wo_q{qt}"):
            obuf = None
            if qt < 3:
                obuf = out_pool.tile([P, 8 * 512], FP16, tag="wo", name="obuf")
            for ot in range(8):
                ps = psum_pv.tile([P, 512], FP32, tag="pv", name="ps_wo")
                for jt in range(2):
                    nc.tensor.matmul(
                        ps[:],
                        woT_s[:, D * jt + P * ot : D * jt + P * ot + P],
                        attnT_q[qt][:, 512 * jt : 512 * jt + 512],
                        start=(jt == 0),
                        stop=(jt == 1),
                    )
                def cp(out, in_):
                    nc.vector.tensor_copy(out=out, in_=in_)

                if qt < 3:
                    cp(out=obuf[:, 512 * ot : 512 * ot + 512], in_=ps[:])
                else:
                    ob = out_pool.tile([P, 512], FP16, tag="wo3", name="ob")
                    cp(out=ob[:], in_=ps[:])
                    nc.sync.dma_start(
                        out=out_dev[P * qt : P * qt + P, 512 * ot : 512 * ot + 512],
                        in_=ob[:],
                    )
            if qt < 3:
                nc.sync.dma_start(out=out_dev[P * qt : P * qt + P, :], in_=obuf[:])

    def vhat_ap(kb, h):
        return vh4[kb // 4][:, VROW * (kb % 4) + HVW * h : VROW * (kb % 4) + HVW * h + 65]

    # interleaved emission: attention on early chunks overlaps later projs;
    # strips for qt3 are emitted before wo_q2 so the last exp chains run on
    # ScalarE while qt2's wo fills the PE
    proj_chunk(0)
    proj_chunk(1)
    strips_qt(0)
    pv_qt(0)
    wo_qt(0)
    proj_chunk(2)
    strips_qt(1)
    pv_qt(1)
    wo_qt(1)
    proj_chunk(3)
    strips_qt(2)
    pv_qt(2)
    strips_qt(3)
    wo_qt(2)
    pv_qt(3)
    wo_qt(3)


_GRAPH_CACHE = {}


def _build(single_stream=True):
    key = ("nc", single_stream)
    if key in _GRAPH_CACHE:
        return _GRAPH_CACHE[key]
    nc = bacc.Bacc("TRN2", target_bir_lowering=False, debug=False, num_devices=NCORES)
    xc = nc.dram_tensor("xc", [4 * P, 8 * 512], BF16, kind="ExternalInput")
    xcg = None
    if not single_stream:
        xcg = nc.dram_tensor("xcg", [4 * P, 8 * 512], BF16, kind="ExternalInput")
    wqkv = nc.dram_tensor("wqkv", [P, 8 * 3 * CW], BF16, kind="ExternalInput")
    woTp = nc.dram_tensor("woTp", [P, 2 * D], BF16, kind="ExternalInput")
    mask2 = nc.dram_tensor("mask2", [P, 2 * P], FP8 if USE_FP8_PV else BF16, kind="ExternalInput")
    ident = nc.dram_tensor("ident", [P, P], BF16, kind="ExternalInput")
    out_dev = nc.dram_tensor("out_dev", [4 * P, 8 * 512], FP16, kind="ExternalOutput")
    from contextlib import ExitStack

    with tile.TileContext(nc) as tc, ExitStack() as ctx:
        _emit(ctx, nc, tc, xc, xcg, wqkv, woTp, mask2, ident, out_dev, single_stream)
    nc.compile()
    _GRAPH_CACHE[key] = nc
    return nc


def _host_masks():
    p = np.arange(P)[:, None]
    c = np.arange(P)[None, :]
    diag = (p <= c).astype(np.float32)   # causal within the diagonal block
    tail = (p > c).astype(np.float32)    # q-k <= 511 within the tail block
    return np.concatenate([diag, tail], axis=1)


def _token_gather(block_tables, slot_mapping):
    """Compose cache scatter (slot_mapping) with block_tables gather into a
    single token index map g[b, t] -> row of x_flat."""
    t = np.arange(S)
    slots = block_tables[:, t // BLOCK].astype(np.int64) * BLOCK + (t % BLOCK)[None, :]
    sm = np.asarray(slot_mapping).astype(np.int64)
    sm_inv = np.empty_like(sm)
    sm_inv[sm] = np.arange(sm.size)
    return sm_inv[slots]  # [B, S]


def _pack_x(x_b, bf):
    # [S, D] f32 -> [4*128, 8*512] packed: row 128*t+p, col 512*dt+c
    # holds x_b[512*t+c, 128*dt+p]; each chunk row is 8KB contiguous.
    return np.ascontiguousarray(
        x_b.reshape(4, 512, 8, P).transpose(0, 3, 2, 1).reshape(4 * P, 8 * 512).astype(bf)
    )


def make_in_maps(x, wq, wk, wv, wo, block_tables, slot_mapping):
    bf = mybir.dt.np(BF16)
    g = _token_gather(np.asarray(block_tables), np.asarray(slot_mapping))
    x_flat = np.ascontiguousarray(np.asarray(x, dtype=np.float32).reshape(B * S, D))
    mask2 = _host_masks().astype(mybir.dt.np(FP8) if USE_FP8_PV else bf)
    ident = np.eye(P, dtype=np.float32).astype(bf)
    wq, wk, wv, wo = (np.asarray(a, dtype=np.float32) for a in (wq, wk, wv, wo))

    single_stream = all(
        np.array_equal(g[bi], np.arange(bi * S, (bi + 1) * S)) for bi in range(B)
    )
    xc_b, xcg_b = [], []
    for bi in range(B):
        xc_b.append(_pack_x(x_flat[bi * S : (bi + 1) * S], bf))
        xcg_b.append(None if single_stream else _pack_x(x_flat[g[bi]], bf))

    in_maps = []
    for c in range(NCORES):
        bi, hg = c // 4, c % 4
        rows = slice(CW * hg, CW * hg + CW)
        wqkvT = np.concatenate([wq[rows].T, wk[rows].T, wv[rows].T], axis=1)
        wqkv_p = np.ascontiguousarray(
            wqkvT.reshape(8, P, 3 * CW).transpose(1, 0, 2).reshape(P, 8 * 3 * CW).astype(bf)
        )
        woT = wo[:, rows].T  # [CW, D]
        woT_p = np.ascontiguousarray(
            woT.reshape(2, P, D).transpose(1, 0, 2).reshape(P, 2 * D).astype(bf)
        )
        m = {
            "xc": xc_b[bi],
            "wqkv": wqkv_p,
            "woTp": woT_p,
            "mask2": mask2,
            "ident": ident,
        }
        if not single_stream:
            m["xcg"] = xcg_b[bi]
        in_maps.append(m)
    return in_maps, single_stream


def kernel(x, wq, wk, wv, wo, block_tables, slot_mapping, context_lens, window_size, **run_kwargs):
    assert int(window_size) == WINDOW, f"kernel hardcodes window {WINDOW}"
    assert tuple(np.asarray(x).shape) == (B, S, D)
    in_maps, single_stream = make_in_maps(x, wq, wk, wv, wo, block_tables, slot_mapping)
    nc = _build(single_stream)
    res = run_bass_kernel_spmd(nc, in_maps, core_ids=list(range(NCORES)), **run_kwargs)
    outs = []
    for r in res.results:
        # out_dev[128*qt+p, 512*ot+c] = outT[128*ot+p, 512*qt+c]
        a = r["out_dev"].astype(np.float32).reshape(4, P, 8, 512)
        outs.append(a.transpose(2, 1, 0, 3).reshape(D, S))
    out = np.stack(
        [sum(outs[4 * bi : 4 * bi + 4]).T for bi in range(B)]
    ).reshape(B, S, D)
    # context_lens == S for these inputs (full visibility); asserted cheaply
    assert np.all(np.asarray(context_lens) == S)
    if run_kwargs:
        kernel.last_result = res
    return out


# revision 30
# speedup vs baseline: 1.0122x; 1.0122x over previous
# Dynamic sparse attention (sliding-window, paged-KV) on 8 TRN2 NeuronCores.
#
# Reference computation (B=2, S=2048, D=1024, H=16, HD=64, window=512):
#   q/k/v = x @ W{q,k,v}.T ; k/v scattered to a paged cache via slot_mapping,
#   gathered back via block_tables ; causal sliding-window attention ;
#   out = attn @ wo.T
#
# Sharding: core c in 0..7 -> batch bi=c//4, head-group hg=c%4 (4 heads each).
# Each core reads only its batch's activations (pre-packed + bf16-cast on
# host) and its head-group's weight slices, and writes a partial output
# (fp16). Host sums the 4 head-group partials per batch and transposes back.
# The paged-cache scatter/gather composes to a single token-gather g
# (identity for the arange block_tables/slot_mapping); it is folded into a
# host-side column gather of x for the K/V projection input.
#
# On-device layout (per core):
#   qT/kT  [128, 2, 2048] bf16   (partition = head-dim pair, free = seq)
#   V^     [128, 16*260] bf16    (keys on partitions; per head 64 V cols +
#                                 a ones column -> PV matmul also accumulates
#                                 the softmax denominator Z for free)
#   scores are computed transposed (S^T[k, q]) per 128-key strip so the
#   exp'd strip feeds the PV matmul directly as the moving operand -- no
#   P transposes. No running-max is needed (scores ~ N(0,1) after 1/8 scale);
#   masked entries are zeroed post-exp by a 0/1 mask multiply on DVE.
#
# Host packs x / weights into chunk-contiguous [128, *] layouts so each
# chunk is one large-line DMA descriptor.

import numpy as np

import concourse.bass as bass
import concourse.tile as tile
from concourse import bacc, mybir
from concourse.bass_utils import run_bass_kernel_spmd

B, S, D, H, HD = 2, 2048, 1024, 16, 64
BLOCK = 16
WINDOW = 512
P = 128
NCORES = 8
HPC = 4          # heads per core
CW = HPC * HD    # per-core projection width = 256
NKB = S // P     # 16 key blocks
NQT = S // 512   # 4 q-tiles of 512
FP32 = mybir.dt.float32
FP16 = mybir.dt.float16
BF16 = mybir.dt.bfloat16
FP8 = mybir.dt.float8e4
HVW = 68         # per-head vhat stride (64 V + 1 ones + 3 pad; 16B-aligned pairs)
VROW = HPC * HVW  # vhat cols per key block
USE_FP8_PV = False


def _strip_width(kb: int) -> int:
    return min(512 + P, S - P * kb)


def _emit(ctx, nc, tc, xc, xcg, wqkv, woTp, mask2, ident, out_dev, single_stream):
    const = ctx.enter_context(tc.tile_pool(name="const", bufs=1))
    xs_pool = ctx.enter_context(tc.tile_pool(name="xs", bufs=3))
    acts = ctx.enter_context(tc.tile_pool(name="acts", bufs=1))
    vt_pool = ctx.enter_context(tc.tile_pool(name="vt", bufs=3))
    strip_pool = ctx.enter_context(tc.tile_pool(name="strips", bufs=20))
    z_pool = ctx.enter_context(tc.tile_pool(name="zch", bufs=4))
    out_pool = ctx.enter_context(tc.tile_pool(name="wo_out", bufs=3))
    psum_mm = ctx.enter_context(tc.tile_pool(name="mm512", bufs=2, space="PSUM"))
    psum_sc = ctx.enter_context(tc.tile_pool(name="pscore", bufs=2, space="PSUM"))
    psum_pv = ctx.enter_context(tc.tile_pool(name="ppv", bufs=2, space="PSUM"))

    # ---- constants (tiles only; DMAs emitted inside proj_chunk(0) so the
    # critical x/weight loads hit the queues first). wqkv is split into
    # per-dt piece tiles so the first proj matmuls start as pieces land. ----
    SDT = FP8 if USE_FP8_PV else BF16  # dtype of exp'd probs, V-hat, mask
    wqkv_p = [const.tile([P, 3 * CW], BF16, name=f"wqkv{dt}") for dt in range(8)]
    woT_s = const.tile([P, 2 * D], BF16, name="woT_s")
    mask_s = const.tile([P, 2 * P], SDT, name="mask_s")
    ident_s = const.tile([P, P], BF16, name="ident_s")

    # ---- per-512-chunk activation tiles: lets attention on early chunks
    # overlap projection of later ones (deps stay per-chunk) ----
    ebias = None
    if USE_FP8_PV:
        ebias = const.tile([P, 1], FP32, name="ebias")
        nc.vector.memset(ebias[:], -3.0)
    qTc = [acts.tile([P, 2 * 512], BF16, name=f"qTc{t}") for t in range(4)]
    kTc = [acts.tile([P, 2 * 512], BF16, name=f"kTc{t}") for t in range(4)]
    vh4 = [acts.tile([P, 4 * VROW], SDT, name=f"vh{t}") for t in range(4)]
    attnT_q = [acts.tile([P, 2 * 512], BF16, name=f"attnT{i}") for i in range(NQT)]
    for t in range(4):
        for h in range(HPC):
            nc.vector.memset(
                vh4[t][:].rearrange("p (kb c) -> p kb c", kb=4)[:, :, HVW * h + 64 : HVW * h + 65],
                1.0,
            )

    def q_ap(h, gc0, gc1):
        # qT slice for global q-cols [gc0, gc1) -- must lie in one chunk
        t = gc0 // 512
        assert gc1 <= 512 * (t + 1)
        ht, hp = h // 2, 64 * (h % 2)
        lo = gc0 - 512 * t
        return qTc[t][hp : hp + 64, 512 * ht + lo : 512 * ht + lo + (gc1 - gc0)]

    def proj_chunk(t):
        with nc.named_scope(f"proj{t}"):
            if t == 0:
                # chunk 0 gates the whole kernel: x pieces and weight pieces
                # are interleaved across both HWDGE queues so the first proj
                # matmuls can start as soon as piece 0 lands; consts follow.
                xs = xs_pool.tile([P, 8 * 512], BF16, tag="xs", name="xs")
                for dt in range(8):
                    xeng = nc.sync if dt % 2 == 0 else nc.scalar
                    xeng.dma_start(
                        out=xs[:, dt * 512 : dt * 512 + 512],
                        in_=xc[0:P, dt * 512 : dt * 512 + 512],
                    )
                    weng = nc.scalar if dt % 2 == 0 else nc.sync
                    weng.dma_start(
                        out=wqkv_p[dt][:],
                        in_=wqkv[:, dt * 3 * CW : (dt + 1) * 3 * CW],
                    )
                nc.scalar.dma_start(out=ident_s[:], in_=ident[:, :])
                nc.scalar.dma_start(out=mask_s[:], in_=mask2[:, :])
                nc.scalar.dma_start(out=woT_s[:], in_=woTp[:, :])
            else:
                xs = xs_pool.tile([P, 8 * 512], BF16, tag="xs", name="xs")
                nc.sync.dma_start(out=xs[:], in_=xc[P * t : P * t + P, :])
            if single_stream:
                xg = xs
            else:
                xg = xs_pool.tile([P, 8 * 512], BF16, tag="xg", name="xg")
                nc.sync.dma_start(out=xg[:], in_=xcg[P * t : P * t + P, :])
            for proj in range(3):
                src = xs if proj == 0 else xg
                for dto in range(2):
                    ps = psum_mm.tile([P, 512], FP32, tag="mm512", name="ps_proj")
                    for dt in range(8):
                        nc.tensor.matmul(
                            ps[:],
                            wqkv_p[dt][:, CW * proj + P * dto : CW * proj + P * dto + P],
                            src[:, dt * 512 : dt * 512 + 512],
                            start=(dt == 0),
                            stop=(dt == 7),
                        )
                    if proj == 0:
                        nc.vector.tensor_copy(out=qTc[t][:, 512 * dto : 512 * dto + 512], in_=ps[:])
                    elif proj == 1:
                        nc.vector.tensor_copy(out=kTc[t][:, 512 * dto : 512 * dto + 512], in_=ps[:])
                    else:
                        vt = vt_pool.tile([P, 512], BF16, tag="vt", name="vt")
                        nc.vector.tensor_copy(out=vt[:], in_=ps[:])
                        for j in range(4):
                            tr = psum_sc.tile([P, P], BF16, tag="score", name="tr")
                            nc.tensor.transpose(tr[:], vt[:, P * j : P * j + P], ident_s[:])
                            vh = vh4[t][:].rearrange(
                                "p (kb g c) -> p kb g c", kb=4, g=HPC
                            )[:, j : j + 1, 2 * dto : 2 * dto + 2, 0:64]
                            nc.any.tensor_copy(
                                out=vh,
                                in_=tr[:].rearrange("p (two c) -> p two c", two=2),
                            )

    # strip pairs: (j, h) -> [128, 2*640] tile holding exp'd scores of key
    # blocks 2j (cols [0,640)) and 2j+1 (cols [640, 640+w)), each in its own
    # q coordinates. Pairing enables fp8 DoubleRow PV matmuls (2 key blocks
    # contracted per pass).
    strip_pairs = {}

    def strip_ap(kb, h, c0, c1):
        # slice of strip kb (local q-cols [c0, c1))
        sp = strip_pairs[(kb // 2, h)]
        off = 640 * (kb % 2)
        return sp[:, off + c0 : off + c1]

    def strip_pair_ap(k0, h, c0, n):
        # [128, 2, n] AP: sub0 = strip k0 cols [c0, c0+n), sub1 = strip k0+1
        # cols [c0-128, c0-128+n) -- the same global q range (offset 128 keys)
        base = strip_pairs[(k0 // 2, h)][:, c0 : c0 + n]
        return bass.AP(base.tensor, base.offset, [base.ap[0], [640 - P, 2], [1, n]])

    def vhat_pair_ap(k0, h):
        base = vh4[k0 // 4][:, VROW * (k0 % 4) + HVW * h : VROW * (k0 % 4) + HVW * h + 65]
        return bass.AP(base.tensor, base.offset, [base.ap[0], [VROW, 2], [1, 65]])

    def strips_qt(qt):
        with nc.named_scope(f"strips_q{qt}"):
            for kb in range(4 * qt, 4 * qt + 4):
                for h in range(HPC):
                    ht, hp = h // 2, 64 * (h % 2)
                    w = _strip_width(kb)
                    n1 = min(512, w)
                    n2 = w - n1
                    ps = psum_sc.tile([P, w], FP32, tag="score", name="ps_sc")
                    lhsT = kTc[kb // 4][hp : hp + 64, 512 * ht + P * (kb % 4) : 512 * ht + P * (kb % 4) + P]
                    # part 1 split at 512-chunk boundaries of qT
                    gc = P * kb
                    while gc < P * kb + n1:
                        end = min(P * kb + n1, (gc // 512 + 1) * 512)
                        nc.tensor.matmul(
                            ps[:, gc - P * kb : end - P * kb],
                            lhsT,
                            q_ap(h, gc, end),
                            start=True,
                            stop=True,
                        )
                        gc = end
                    if n2:
                        nc.tensor.matmul(
                            ps[:, 512 : 512 + n2],
                            lhsT,
                            q_ap(h, P * kb + 512, P * kb + 512 + n2),
                            start=True,
                            stop=True,
                        )
                    if kb % 2 == 0:
                        strip_pairs[(kb // 2, h)] = strip_pool.tile(
                            [P, 2 * 640], SDT, tag="strip", name="strip"
                        )
                    st = strip_ap(kb, h, 0, w)
                    # fp8: bias the exponent down so exp stays within e4m3
                    # range (max |score| ~ 5.5 -> e^3.5 = 33 < 240); the
                    # softmax 1/Z cancels the constant factor exactly
                    nc.scalar.activation(
                        st, ps[:], mybir.ActivationFunctionType.Exp,
                        scale=float(HD) ** -0.5,
                        bias=ebias[:] if USE_FP8_PV else 0.0,
                    )
                    if n2 == P:
                        ed = st.rearrange("p (a c) -> p a c", c=P)[:, 0:5:4, :]
                        nc.vector.tensor_mul(
                            out=ed, in0=ed, in1=mask_s[:].rearrange("p (a c) -> p a c", c=P)
                        )
                    else:
                        nc.vector.tensor_mul(out=st[:, 0:P], in0=st[:, 0:P], in1=mask_s[:, 0:P])
                        if n2:
                            nc.vector.tensor_mul(
                                out=st[:, 512 : 512 + n2],
                                in0=st[:, 512 : 512 + n2],
                                in1=mask_s[:, P : P + n2],
                            )

    def pv_qt(qt):
        with nc.named_scope(f"pv_q{qt}"):
            W0 = 512 * qt
            for h in range(HPC):
                ht, hp = h // 2, 64 * (h % 2)
                pv = psum_pv.tile([65, 512], FP32, tag="pv", name="ps_pv")
                nc.vector.memset(pv[:], 0.0)
                for k0 in range(max(0, 4 * qt - 4), 4 * qt + 4, 2):
                    k1 = k0 + 1
                    c0a = max(P * k0, W0)
                    c0b = min(P * k0 + _strip_width(k0), W0 + 512)
                    c1a = max(P * k1, W0)
                    c1b = min(P * k1 + _strip_width(k1), W0 + 512)
                    a, b = max(c0a, c1a), min(c0b, c1b)
                    if USE_FP8_PV and b > a:
                        nc.tensor.matmul(
                            pv[:, a - W0 : b - W0],
                            vhat_pair_ap(k0, h),
                            strip_pair_ap(k0, h, a - P * k0, b - a),
                            start=False,
                            stop=False,
                            perf_mode=mybir.MatmulPerfMode.DoubleRow,
                            skip_group_check=True,
                        )
                        singles = [(k0, c0a, min(a, c0b)), (k1, max(b, c1a), c1b)]
                    else:
                        singles = [(k0, c0a, c0b), (k1, c1a, c1b)]
                    for kk, sa, sb in singles:
                        if sb <= sa:
                            continue
                        nc.tensor.matmul(
                            pv[:, sa - W0 : sb - W0],
                            vhat_ap(kk, h),
                            strip_ap(kk, h, sa - P * kk, sb - P * kk),
                            start=False,
                            stop=False,
                            skip_group_check=True,
                        )
                zs = z_pool.tile([1, 512], FP32, tag="zs", name="zs")
                nc.scalar.copy(out=zs[:], in_=pv[64:65, :])
                zr = z_pool.tile([1, 512], FP32, tag="zr", name="zr")
                nc.vector.reciprocal_approx_fast(out=zr[:], in_=zs[:])
                zrb = z_pool.tile([64, 512], FP32, tag="zrb", name="zrb")
                nc.gpsimd.partition_broadcast(zrb[:], zr[:])
                nc.vector.tensor_mul(
                    out=attnT_q[qt][hp : hp + 64, 512 * ht : 512 * ht + 512],
                    in0=pv[0:64, :],
                    in1=zrb[:],
                )

    def wo_qt(qt):
        with nc.named_scope(f"wo_q{qt}"):
            obuf = None
            if qt < 3:
                obuf = out_pool.tile([P, 8 * 512], FP16, tag="wo", name="obuf")
            for ot in range(8):
                ps = psum_pv.tile([P, 512], FP32, tag="pv", name="ps_wo")
                for jt in range(2):
                    nc.tensor.matmul(
                        ps[:],
                        woT_s[:, D * jt + P * ot : D * jt + P * ot + P],
                        attnT_q[qt][:, 512 * jt : 512 * jt + 512],
                        start=(jt == 0),
                        stop=(jt == 1),
                    )
                def cp(out, in_):
                    nc.vector.tensor_copy(out=out, in_=in_)

                if qt < 3:
                    cp(out=obuf[:, 512 * ot : 512 * ot + 512], in_=ps[:])
                else:
                    ob = out_pool.tile([P, 512], FP16, tag="wo3", name="ob")
                    cp(out=ob[:], in_=ps[:])
                    nc.sync.dma_start(
                        out=out_dev[P * qt : P * qt + P, 512 * ot : 512 * ot + 512],
                        in_=ob[:],
                    )
            if qt < 3:
                nc.sync.dma_start(out=out_dev[P * qt : P * qt + P, :], in_=obuf[:])

    def vhat_ap(kb, h):
        return vh4[kb // 4][:, VROW * (kb % 4) + HVW * h : VROW * (kb % 4) + HVW * h + 65]

    # interleaved emission: attention on early chunks overlaps later projs;
    # strips for qt3 are emitted before wo_q2 so the last exp chains run on
    # ScalarE while qt2's wo fills the PE
    proj_chunk(0)
    proj_chunk(1)
    strips_qt(0)
    pv_qt(0)
    wo_qt(0)
    proj_chunk(2)
    strips_qt(1)
    pv_qt(1)
    wo_qt(1)
    proj_chunk(3)
    strips_qt(2)
    pv_qt(2)
    strips_qt(3)
    wo_qt(2)
    pv_qt(3)
    wo_qt(3)


_GRAPH_CACHE = {}


def _build(single_stream=True):
    key = ("nc", single_stream)
    if key in _GRAPH_CACHE:
        return _GRAPH_CACHE[key]
    nc = bacc.Bacc("TRN2", target_bir_lowering=False, debug=False, num_devices=NCORES)
    xc = nc.dram_tensor("xc", [4 * P, 8 * 512], BF16, kind="ExternalInput")
    xcg = None
    if not single_stream:
        xcg = nc.dram_tensor("xcg", [4 * P, 8 * 512], BF16, kind="ExternalInput")
    wqkv = nc.dram_tensor("wqkv", [P, 8 * 3 * CW], BF16, kind="ExternalInput")
    woTp = nc.dram_tensor("woTp", [P, 2 * D], BF16, kind="ExternalInput")
    mask2 = nc.dram_tensor("mask2", [P, 2 * P], FP8 if USE_FP8_PV else BF16, kind="ExternalInput")
    ident = nc.dram_tensor("ident", [P, P], BF16, kind="ExternalInput")
    out_dev = nc.dram_tensor("out_dev", [4 * P, 8 * 512], FP16, kind="ExternalOutput")
    from contextlib import ExitStack

    with tile.TileContext(nc) as tc, ExitStack() as ctx:
        _emit(ctx, nc, tc, xc, xcg, wqkv, woTp, mask2, ident, out_dev, single_stream)
    nc.compile()
    _GRAPH_CACHE[key] = nc
    return nc


def _host_masks():
    p = np.arange(P)[:, None]
    c = np.arange(P)[None, :]
    diag = (p <= c).astype(np.float32)   # causal within the diagonal block
    tail = (p > c).astype(np.float32)    # q-k <= 511 within the tail block
    return np.concatenate([diag, tail], axis=1)


def _token_gather(block_tables, slot_mapping):
    """Compose cache scatter (slot_mapping) with block_tables gather into a
    single token index map g[b, t] -> row of x_flat."""
    t = np.arange(S)
    slots = block_tables[:, t // BLOCK].astype(np.int64) * BLOCK + (t % BLOCK)[None, :]
    sm = np.asarray(slot_mapping).astype(np.int64)
    sm_inv = np.empty_like(sm)
    sm_inv[sm] = np.arange(sm.size)
    return sm_inv[slots]  # [B, S]


def _pack_x(x_b, bf):
    # [S, D] f32 -> [4*128, 8*512] packed: row 128*t+p, col 512*dt+c
    # holds x_b[512*t+c, 128*dt+p]; each chunk row is 8KB contiguous.
    return np.ascontiguousarray(
        x_b.reshape(4, 512, 8, P).transpose(0, 3, 2, 1).reshape(4 * P, 8 * 512).astype(bf)
    )


def make_in_maps(x, wq, wk, wv, wo, block_tables, slot_mapping):
    bf = mybir.dt.np(BF16)
    g = _token_gather(np.asarray(block_tables), np.asarray(slot_mapping))
    x_flat = np.ascontiguousarray(np.asarray(x, dtype=np.float32).reshape(B * S, D))
    mask2 = _host_masks().astype(mybir.dt.np(FP8) if USE_FP8_PV else bf)
    ident = np.eye(P, dtype=np.float32).astype(bf)
    wq, wk, wv, wo = (np.asarray(a, dtype=np.float32) for a in (wq, wk, wv, wo))

    single_stream = all(
        np.array_equal(g[bi], np.arange(bi * S, (bi + 1) * S)) for bi in range(B)
    )
    xc_b, xcg_b = [], []
    for bi in range(B):
        xc_b.append(_pack_x(x_flat[bi * S : (bi + 1) * S], bf))
        xcg_b.append(None if single_stream else _pack_x(x_flat[g[bi]], bf))

    in_maps = []
    for c in range(NCORES):
        bi, hg = c // 4, c % 4
        rows = slice(CW * hg, CW * hg + CW)
        wqkvT = np.concatenate([wq[rows].T, wk[rows].T, wv[rows].T], axis=1)
        wqkv_p = np.ascontiguousarray(
            wqkvT.reshape(8, P, 3 * CW).transpose(1, 0, 2).reshape(P, 8 * 3 * CW).astype(bf)
        )
        woT = wo[:, rows].T  # [CW, D]
        woT_p = np.ascontiguousarray(
            woT.reshape(2, P, D).transpose(1, 0, 2).reshape(P, 2 * D).astype(bf)
        )
        m = {
            "xc": xc_b[bi],
            "wqkv": wqkv_p,
            "woTp": woT_p,
            "mask2": mask2,
            "ident": ident,
        }
        if not single_stream:
            m["xcg"] = xcg_b[bi]
        in_maps.append(m)
    return in_maps, single_stream


def kernel(x, wq, wk, wv, wo, block_tables, slot_mapping, context_lens, window_size, **run_kwargs):
    assert int(window_size) == WINDOW, f"kernel hardcodes window {WINDOW}"
    assert tuple(np.asarray(x).shape) == (B, S, D)
    in_maps, single_stream = make_in_maps(x, wq, wk, wv, wo, block_tables, slot_mapping)
    nc = _build(single_stream)
    res = run_bass_kernel_spmd(nc, in_maps, core_ids=list(range(NCORES)), **run_kwargs)
    outs = []
    for r in res.results:
        # out_dev[128*qt+p, 512*ot+c] = outT[128*ot+p, 512*qt+c]
        a = r["out_dev"].astype(np.float32).reshape(4, P, 8, 512)
        outs.append(a.transpose(2, 1, 0, 3).reshape(D, S))
    out = np.stack(
        [sum(outs[4 * bi : 4 * bi + 4]).T for bi in range(B)]
    ).reshape(B, S, D)
    # context_lens == S for these inputs (full visibility); asserted cheaply
    assert np.all(np.asarray(context_lens) == S)
    if run_kwargs:
        kernel.last_result = res
    return out
